# revision 6
# baseline (speedup 1.0000x reference)
"""Multi-head causal attention (B=8, T=1024, C=1024, H=16, hs=64) on 8 trn2 cores.

Data-parallel over batch: core b computes full attention for x[b].

Device algorithm (per core), all matmuls bf16 inputs / fp32 PSUM accum:
  - xT [C, T] resident in SBUF as 8 per-chunk tiles (fine-grained DMA deps).
  - v computed for all heads: v_all[s, head, s_tile, 0:64] + ones column at 64
    so the AV matmul also produces softmax denominators.
  - per head-pair: qT, kT = W^T @ xT -> [128, T] (2 heads x 64 stacked).
  - scores transposed scT[s_tile, t] = kT_chunk^T @ qT, both heads concurrent
    in PE row groups (0,0)/(64,0); exp fused on ScalarE with ONE activation
    per span covering both heads (3D APs into a merged [128, 2, T] exp tile);
    diagonal 128x128 masked by one two-head tril multiply on VectorE.
  - out^T[65, t] accumulated over s chunks per head (lhsT = [v | 1]); row 64 is
    the softmax denominator. Normalize: DVE fast-reciprocal of the PSUM
    denominator row -> GpSimd partition-broadcast -> one DVE multiply reading
    the AV PSUM directly -> DMA out (issued from the GpSimd queue).

Scheduling: per-engine queues execute in emission order, so the program is
emitted as a software pipeline: slot p interleaves scores(p) (paced against
the ScalarE exp stream via the single-score-PSUM-tile WAR dependency) with
QK-projection chunks of pair p+1 and the AV+normalize of pair p-1 as PE
filler. V-projection chunks fill slot 0. This keeps the PE dense (no HAM
re-throttle) from ~1.5us after launch to the tail.
"""

from collections import deque

import numpy as np
import ml_dtypes

import concourse.bass as bass
import concourse.mybir as mybir
from concourse import bacc
from concourse.tile import TileContext
from concourse.bass import ds, ts
from concourse.bass_utils import run_bass_kernel_spmd
from concourse.masks import make_upper_triangular

BF16 = mybir.dt.bfloat16
F32 = mybir.dt.float32
MULT = mybir.AluOpType.mult
EXP = mybir.ActivationFunctionType.Exp

B, T, C, H, HS = 8, 1024, 1024, 16, 64
P = 128
CK = C // P       # 8 contraction chunks
TT = T // P       # 8 s tiles
PAIRS = H // 2    # 8 head pairs
HALF = 512

_BUILT = None


def _spans(i):
    t0 = P * i
    return [(t0, HALF), (HALF, T)] if t0 < HALF else [(t0, T)]


def _exp_us(i):
    """ScalarE time (us) for tile i's exp instructions (pacing budget)."""
    return sum(2 * (b - a) + 352 for a, b in _spans(i)) / 1200.0


def build_nc():
    nc = bacc.Bacc("TRN2", target_bir_lowering=False, debug=False)
    # [p, c, t] : xT[C, T] chunked; partition p, chunk c -> row 128c+p of xT
    xt = nc.dram_tensor("xt", [P, CK, T], BF16, kind="ExternalInput")
    # [proj(q,k), pair, p, c, f] : lhsT chunks, f = 2 heads x 64 stacked
    wqk = nc.dram_tensor("wqk", [2, PAIRS, P, CK, P], BF16, kind="ExternalInput")
    # [p, c, pair, f]
    wv = nc.dram_tensor("wv", [P, CK, PAIRS, P], BF16, kind="ExternalInput")
    # out^T per head: [head, d, t]; host transposes to [T, H*HS]
    out = nc.dram_tensor("out", [H, HS, T], F32, kind="ExternalOutput")

    with TileContext(nc) as tc:
        with (
            tc.tile_pool(name="const", bufs=1) as constp,
            tc.tile_pool(name="wpool", bufs=8) as wpool,
            tc.tile_pool(name="qkpool", bufs=6) as qkp,
            tc.tile_pool(name="exppool", bufs=18) as expp,
            tc.tile_pool(name="smallpool", bufs=4) as smallp,
            tc.tile_pool(name="psProj", bufs=3, space="PSUM") as psProj,
            tc.tile_pool(name="psSc", bufs=1, space="PSUM") as psSc,
            tc.tile_pool(name="psAV", bufs=3, space="PSUM") as psAV,
        ):
            # ---- input DMAs (sync queue, ordered for earliest PE start) ----
            wq_sb = [None] * PAIRS
            wk_sb = [None] * PAIRS

            def load_w(pair, which):
                t = wpool.tile([P, CK, P], BF16, tag="w",
                               name=f"w{'qk'[which]}{pair}")
                nc.sync.dma_start(t[:, :, :], wqk[which, pair, :, :, :])
                (wq_sb if which == 0 else wk_sb)[pair] = t

            load_w(0, 0)
            xt_c = []
            for c in range(CK):
                t = constp.tile([P, T], BF16, tag=f"xt{c}", name=f"xt{c}")
                nc.sync.dma_start(t[:, :], xt[:, c, :])
                xt_c.append(t)
            load_w(0, 1)
            wv_c = []
            for c in range(CK):
                t = constp.tile([P, 2, 4 * P], BF16, tag=f"wv{c}", name=f"wv{c}")
                nc.sync.dma_start(
                    t[:, :, :],
                    wv[:, c, :, :].rearrange("p (g r) f -> p g (r f)", g=2),
                )
                wv_c.append(t)
            load_w(1, 0)
            load_w(1, 1)
            load_w(2, 0)
            load_w(2, 1)

            # two-head tril mask [p, w, t'] and v (+ones) for all heads
            mask2 = constp.tile([P, 2, P], BF16, tag="mask", name="mask2")
            for w in range(2):
                make_upper_triangular(nc, mask2[:, w, :], val=1.0, diag=True)
            v_all = constp.tile([P, H, TT, HS + 1], BF16, tag="vall", name="v_all")
            nc.gpsimd.memset(v_all[:, :, :, HS:HS + 1], 1.0)

            qT = [None] * PAIRS
            kT = [None] * PAIRS
            es_tiles = [[None] * TT for _ in range(PAIRS)]
            av_tiles = {}

            # ---- emission units ----
            def proj_units(pair, which):
                """QK projection for one proj: 8 c-units (LDW + 2 MMs) + cast."""
                holder = {}

                def mm(c):
                    if c == 0:
                        wsb = (wq_sb if which == 0 else wk_sb)[pair]
                        dst = qkp.tile([P, T], BF16, tag="qk",
                                       name=f"{'qk'[which]}T{pair}")
                        (qT if which == 0 else kT)[pair] = dst
                        holder['w'] = wsb
                        holder['dst'] = dst
                        holder['ps'] = [
                            psProj.tile([P, HALF], F32, tag="ps",
                                        name=f"p{'qk'[which]}{pair}_{g}")
                            for g in range(2)
                        ]
                    for g in range(2):
                        nc.tensor.matmul(
                            holder['ps'][g][:, :],
                            holder['w'][:, c, :],
                            xt_c[c][:, ds(HALF * g, HALF)],
                            start=(c == 0),
                            stop=(c == CK - 1),
                        )

                def cast():
                    for g in range(2):
                        nc.vector.tensor_copy(
                            holder['dst'][:, ds(HALF * g, HALF)],
                            holder['ps'][g][:, :])

                return [(0.45, lambda c=c: mm(c)) for c in range(CK)] + \
                       [(0.02, cast)]

            def v_unit(j):
                def f():
                    pvs = [psProj.tile([P, HALF], F32, tag="ps",
                                       name=f"pv{j}_{g}") for g in range(2)]
                    for c in range(CK):
                        for pg in range(2):
                            nc.tensor.matmul(
                                pvs[pg][:, :],
                                xt_c[c][:, ts(j, P)],
                                wv_c[c][:, pg, :],
                                start=(c == 0),
                                stop=(c == CK - 1),
                            )
                    for pg in range(2):
                        nc.vector.tensor_copy(
                            v_all[:, ds(8 * pg, 8), j, 0:HS],
                            pvs[pg].rearrange("p (g d) -> p g d", d=HS),
                        )
                return f

            def scores_tile(pair, i):
                t0 = P * i
                es = expp.tile([P, 2, T], BF16, tag="exp", name=f"es{pair}_{i}")
                es_tiles[pair][i] = es
                sc = psSc.tile([P, T], F32, tag="sc", name=f"sc{pair}_{i}")
                scv = sc.rearrange("p (w t) -> p w t", w=2)
                for a, b in _spans(i):
                    L = b - a
                    for w in range(2):
                        nc.tensor.matmul(
                            scv[:, w, 0:L],
                            kT[pair][ds(HS * w, HS), ds(t0, P)],
                            qT[pair][ds(HS * w, HS), ds(a, L)],
                        )
                    nc.scalar.activation(
                        es[:, :, ds(a, L)], scv[:, :, 0:L], EXP,
                        scale=HS ** -0.5,
                    )
                    if a == t0:  # diagonal block lives in the first span
                        nc.vector.tensor_tensor(
                            es[:, :, ds(t0, P)], es[:, :, ds(t0, P)],
                            mask2[:, :, :], MULT,
                        )

            def av_unit(pair, hh):
                def f():
                    avs = [psAV.tile([HS + 1, HALF], F32, tag="av",
                                     name=f"av{pair}_{hh}_{w}") for w in range(2)]
                    av_tiles[(pair, hh)] = avs
                    contrib = [i for i in range(TT) if P * i < HALF * (hh + 1)]
                    for idx, i in enumerate(contrib):
                        g0 = max(HALF * hh, P * i)
                        g1 = HALF * (hh + 1)
                        for w in range(2):
                            nc.tensor.matmul(
                                avs[w][:, ds(g0 - HALF * hh, g1 - g0)],
                                v_all[:, 2 * pair + w, i, :],
                                es_tiles[pair][i][:, w, ds(g0, g1 - g0)],
                                start=(idx == 0),
                                stop=(idx == len(contrib) - 1),
                            )
                return f

            def norm_unit(pair, hh):
                def f():
                    avs = av_tiles[(pair, hh)]
                    rbs = []
                    for w in range(2):
                        rr = smallp.tile([1, HALF], F32, tag="rr",
                                         name=f"rr{pair}_{hh}_{w}")
                        nc.vector.reciprocal(rr[:, :], avs[w][HS:HS + 1, :])
                        rb = smallp.tile([HS, HALF], F32, tag="rb",
                                         name=f"rb{pair}_{hh}_{w}")
                        nc.gpsimd.partition_broadcast(rb[:, :], rr[0:1, :])
                        rbs.append(rb)
                    for w in range(2):
                        h = 2 * pair + w
                        osb = smallp.tile([HS, HALF], F32, tag="osb",
                                          name=f"osb{pair}_{hh}_{w}")
                        nc.vector.tensor_tensor(
                            osb[:, :], avs[w][0:HS, :], rbs[w][:, :], MULT)
                        nc.gpsimd.dma_start(
                            out[h, :, ds(HALF * hh, HALF)], osb[:, :])
                return f

            # ---- slot 0: QK proj pair0, then scores(0) with V as filler ----
            for _, u in proj_units(0, 0) + proj_units(0, 1):
                u()
            for i in range(TT):
                scores_tile(0, i)
                v_unit(i)()
            for _, u in proj_units(1, 0) + proj_units(1, 1):
                u()

            # ---- slots 1..7 ----
            for s in range(1, PAIRS):
                if s + 2 < PAIRS:
                    load_w(s + 2, 0)
                    load_w(s + 2, 1)
                fill = deque()
                if s + 1 < PAIRS:
                    fill.extend(proj_units(s + 1, 0))
                fill.append((1.1, av_unit(s - 1, 0)))
                fill.append((0.05, norm_unit(s - 1, 0)))
                if s + 1 < PAIRS:
                    fill.extend(proj_units(s + 1, 1))
                fill.append((2.8, av_unit(s - 1, 1)))
                fill.append((0.05, norm_unit(s - 1, 1)))
                for i in range(TT):
                    scores_tile(s, i)
                    if s == PAIRS - 1 and i == 5:
                        # last pair: its first-half AV only needs es tiles
                        # 0..3, all emitted by now — overlap it with the
                        # remaining exp stream instead of the drain.
                        av_unit(s, 0)()
                        norm_unit(s, 0)()
                    budget = _exp_us(i)
                    while fill and budget > 0:
                        cost, u = fill.popleft()
                        u()
                        budget -= cost
                while fill:
                    fill.popleft()[1]()

            # ---- drain: last pair's second half ----
            av_unit(PAIRS - 1, 1)()
            norm_unit(PAIRS - 1, 1)()

    nc.compile()
    return nc


def get_nc():
    global _BUILT
    if _BUILT is None:
        _BUILT = build_nc()
    return _BUILT


def prep_inputs(x, Wq, Wk, Wv):
    """Host-side shard + layout prep. Returns in_maps (one dict per core)."""
    x = np.asarray(x, dtype=np.float32)
    Wq = np.asarray(Wq, dtype=np.float32)
    Wk = np.asarray(Wk, dtype=np.float32)
    Wv = np.asarray(Wv, dtype=np.float32)
    bf = ml_dtypes.bfloat16

    # xT[b]: [C, T] -> [p, c, t] with row 128c+p
    xts = []
    for b in range(B):
        xT = np.ascontiguousarray(x[b].T)          # [C, T]
        xts.append(xT.reshape(CK, P, T).transpose(1, 0, 2).astype(bf))

    def pack_pairs(W):
        # [H, C, hs] -> [pair, C, 128] -> [pair, p, c, f]
        Wp = W.reshape(PAIRS, 2, C, HS).transpose(0, 2, 1, 3).reshape(PAIRS, C, P)
        return Wp.reshape(PAIRS, CK, P, P).transpose(0, 2, 1, 3)  # [pair, p, c, f]

    wq_p = pack_pairs(Wq)
    wk_p = pack_pairs(Wk)
    wqk_host = np.stack([wq_p, wk_p], axis=0).astype(bf)  # [2, pair, p, c, f]
    # wv: [p, c, pair, f]
    wv_host = np.ascontiguousarray(pack_pairs(Wv).transpose(1, 2, 0, 3)).astype(bf)

    return [
        {"xt": np.ascontiguousarray(xts[b]), "wqk": wqk_host, "wv": wv_host}
        for b in range(B)
    ]


def run_on_device(in_maps, **kwargs):
    nc = get_nc()
    return run_bass_kernel_spmd(nc, in_maps, list(range(B)), **kwargs)


def assemble(core_out):
    """[H, HS, T] out^T -> [T, H*HS]: pure layout transpose."""
    return np.ascontiguousarray(core_out.transpose(2, 0, 1).reshape(T, H * HS))


def kernel(x, Wq, Wk, Wv):
    in_maps = prep_inputs(x, Wq, Wk, Wv)
    res = run_on_device(in_maps)
    return np.stack([assemble(res.results[b]["out"]) for b in range(B)], axis=0)


# revision 7
# speedup vs baseline: 1.3917x; 1.3917x over previous
"""Multi-head causal attention (B=8, T=1024, C=1024, H=16, hs=64) on 8 trn2 cores.

Data-parallel over batch: core b computes full attention for x[b].

Device algorithm (per core), all matmuls bf16 inputs / fp32 PSUM accum:
  - xT [C, T] resident in SBUF as 8 per-chunk tiles (fine-grained DMA deps).
  - v computed for all heads: v_all[s, head, s_tile, 0:64] + ones column at 64
    so the AV matmul also produces softmax denominators.
  - per head-pair: qT, kT = W^T @ xT -> [128, T] (2 heads x 64 stacked).
  - scores transposed scT[s_tile, t] = kT_chunk^T @ qT, both heads concurrent
    in PE row groups (0,0)/(64,0); exp fused on ScalarE with ONE activation
    per span covering both heads (3D APs into a merged [128, 2, T] exp tile);
    diagonal 128x128 masked by one two-head tril multiply on VectorE.
  - out^T[65, t] accumulated over s chunks per head (lhsT = [v | 1]); row 64 is
    the softmax denominator. Normalize: DVE fast-reciprocal of the PSUM
    denominator row -> GpSimd partition-broadcast -> one DVE multiply reading
    the AV PSUM directly -> DMA out (issued from the GpSimd queue).

Scheduling: per-engine queues execute in emission order, so the program is
emitted as a software pipeline: slot p interleaves scores(p) (paced against
the ScalarE exp stream via the single-score-PSUM-tile WAR dependency) with
QK-projection chunks of pair p+1 and the AV+normalize of pair p-1 as PE
filler. V-projection chunks fill slot 0. This keeps the PE dense (no HAM
re-throttle) from ~1.5us after launch to the tail.
"""

from collections import deque

import numpy as np
import ml_dtypes

import concourse.bass as bass
import concourse.mybir as mybir
from concourse import bacc
from concourse.tile import TileContext
from concourse.bass import ds, ts
from concourse.bass_utils import run_bass_kernel_spmd
from concourse.masks import make_upper_triangular

BF16 = mybir.dt.bfloat16
F32 = mybir.dt.float32
MULT = mybir.AluOpType.mult
EXP = mybir.ActivationFunctionType.Exp

B, T, C, H, HS = 8, 1024, 1024, 16, 64
P = 128
CK = C // P       # 8 contraction chunks
TT = T // P       # 8 s tiles
PAIRS = H // 2    # 8 head pairs
HALF = 512

_BUILT = None


def _spans(i):
    t0 = P * i
    return [(t0, HALF), (HALF, T)] if t0 < HALF else [(t0, T)]


def _exp_us(i):
    """ScalarE time (us) for tile i's exp instructions (pacing budget)."""
    return sum(2 * (b - a) + 352 for a, b in _spans(i)) / 1200.0


def build_nc():
    nc = bacc.Bacc("TRN2", target_bir_lowering=False, debug=False)
    # [p, c, t] : xT[C, T] chunked; partition p, chunk c -> row 128c+p of xT
    xt = nc.dram_tensor("xt", [P, CK, T], BF16, kind="ExternalInput")
    # [proj(q,k), pair, p, c, f] : lhsT chunks, f = 2 heads x 64 stacked
    wqk = nc.dram_tensor("wqk", [2, PAIRS, P, CK, P], BF16, kind="ExternalInput")
    # [p, c, pair, f]
    wv = nc.dram_tensor("wv", [P, CK, PAIRS, P], BF16, kind="ExternalInput")
    # out^T per head: [head, d, t]; host transposes to [T, H*HS]
    out = nc.dram_tensor("out", [H, HS, T], F32, kind="ExternalOutput")

    with TileContext(nc) as tc:
        with (
            tc.tile_pool(name="const", bufs=1) as constp,
            tc.tile_pool(name="wpool", bufs=8) as wpool,
            tc.tile_pool(name="qkpool", bufs=6) as qkp,
            tc.tile_pool(name="exppool", bufs=18) as expp,
            tc.tile_pool(name="smallpool", bufs=4) as smallp,
            tc.tile_pool(name="psProj", bufs=3, space="PSUM") as psProj,
            tc.tile_pool(name="psSc", bufs=1, space="PSUM") as psSc,
            tc.tile_pool(name="psAV", bufs=3, space="PSUM") as psAV,
        ):
            # ---- input DMAs (sync queue, ordered for earliest PE start) ----
            wq_sb = [None] * PAIRS
            wk_sb = [None] * PAIRS

            def load_w(pair, which):
                t = wpool.tile([P, CK, P], BF16, tag="w",
                               name=f"w{'qk'[which]}{pair}")
                nc.sync.dma_start(t[:, :, :], wqk[which, pair, :, :, :])
                (wq_sb if which == 0 else wk_sb)[pair] = t

            load_w(0, 0)
            xt_c = []
            for c in range(CK):
                t = constp.tile([P, T], BF16, tag=f"xt{c}", name=f"xt{c}")
                nc.sync.dma_start(t[:, :], xt[:, c, :])
                xt_c.append(t)
            load_w(0, 1)
            wv_c = []
            for c in range(CK):
                t = constp.tile([P, 2, 4 * P], BF16, tag=f"wv{c}", name=f"wv{c}")
                nc.sync.dma_start(
                    t[:, :, :],
                    wv[:, c, :, :].rearrange("p (g r) f -> p g (r f)", g=2),
                )
                wv_c.append(t)
            load_w(1, 0)
            load_w(1, 1)
            load_w(2, 0)
            load_w(2, 1)

            # two-head tril mask [p, w, t'] and v (+ones) for all heads
            mask2 = constp.tile([P, 2, P], BF16, tag="mask", name="mask2")
            for w in range(2):
                make_upper_triangular(nc, mask2[:, w, :], val=1.0, diag=True)
            v_all = constp.tile([P, H, TT, HS + 1], BF16, tag="vall", name="v_all")
            nc.gpsimd.memset(v_all[:, :, :, HS:HS + 1], 1.0)

            qT = [None] * PAIRS
            kT = [None] * PAIRS
            es_tiles = [[None] * TT for _ in range(PAIRS)]
            av_tiles = {}

            # ---- emission units ----
            def proj_units(pair, which):
                """QK projection for one proj: 8 c-units (LDW + 2 MMs) + cast."""
                holder = {}

                def mm(c):
                    if c == 0:
                        wsb = (wq_sb if which == 0 else wk_sb)[pair]
                        dst = qkp.tile([P, T], BF16, tag="qk",
                                       name=f"{'qk'[which]}T{pair}")
                        (qT if which == 0 else kT)[pair] = dst
                        holder['w'] = wsb
                        holder['dst'] = dst
                        holder['ps'] = [
                            psProj.tile([P, HALF], F32, tag="ps",
                                        name=f"p{'qk'[which]}{pair}_{g}")
                            for g in range(2)
                        ]
                    for g in range(2):
                        nc.tensor.matmul(
                            holder['ps'][g][:, :],
                            holder['w'][:, c, :],
                            xt_c[c][:, ds(HALF * g, HALF)],
                            start=(c == 0),
                            stop=(c == CK - 1),
                        )

                def cast():
                    for g in range(2):
                        nc.vector.tensor_copy(
                            holder['dst'][:, ds(HALF * g, HALF)],
                            holder['ps'][g][:, :])

                return [(0.45, lambda c=c: mm(c)) for c in range(CK)] + \
                       [(0.02, cast)]

            def v_unit(j):
                def f():
                    pvs = [psProj.tile([P, HALF], F32, tag="ps",
                                       name=f"pv{j}_{g}") for g in range(2)]
                    for c in range(CK):
                        for pg in range(2):
                            nc.tensor.matmul(
                                pvs[pg][:, :],
                                xt_c[c][:, ts(j, P)],
                                wv_c[c][:, pg, :],
                                start=(c == 0),
                                stop=(c == CK - 1),
                            )
                    for pg in range(2):
                        nc.vector.tensor_copy(
                            v_all[:, ds(8 * pg, 8), j, 0:HS],
                            pvs[pg].rearrange("p (g d) -> p g d", d=HS),
                        )
                return f

            def scores_tile(pair, i):
                t0 = P * i
                es = expp.tile([P, 2, T], BF16, tag="exp", name=f"es{pair}_{i}")
                es_tiles[pair][i] = es
                sc = psSc.tile([P, T], F32, tag="sc", name=f"sc{pair}_{i}")
                scv = sc.rearrange("p (w t) -> p w t", w=2)
                for a, b in _spans(i):
                    L = b - a
                    for w in range(2):
                        nc.tensor.matmul(
                            scv[:, w, 0:L],
                            kT[pair][ds(HS * w, HS), ds(t0, P)],
                            qT[pair][ds(HS * w, HS), ds(a, L)],
                        )
                    nc.scalar.activation(
                        es[:, :, ds(a, L)], scv[:, :, 0:L], EXP,
                        scale=HS ** -0.5,
                    )
                    if a == t0:  # diagonal block lives in the first span
                        nc.vector.tensor_tensor(
                            es[:, :, ds(t0, P)], es[:, :, ds(t0, P)],
                            mask2[:, :, :], MULT,
                        )

            def av_unit(pair, hh):
                def f():
                    avs = [psAV.tile([HS + 1, HALF], F32, tag="av",
                                     name=f"av{pair}_{hh}_{w}") for w in range(2)]
                    av_tiles[(pair, hh)] = avs
                    contrib = [i for i in range(TT) if P * i < HALF * (hh + 1)]
                    for idx, i in enumerate(contrib):
                        g0 = max(HALF * hh, P * i)
                        g1 = HALF * (hh + 1)
                        for w in range(2):
                            nc.tensor.matmul(
                                avs[w][:, ds(g0 - HALF * hh, g1 - g0)],
                                v_all[:, 2 * pair + w, i, :],
                                es_tiles[pair][i][:, w, ds(g0, g1 - g0)],
                                start=(idx == 0),
                                stop=(idx == len(contrib) - 1),
                            )
                return f

            def norm_unit(pair, hh):
                """Stage AV PSUM out fast (frees the psAV slot), then
                normalize off SBUF: DMA-repartition the [1,512] denominator
                row to [128,4] (DVE reciprocal cost scales with per-lane
                free size), recip, DMA back, GpSimd partition-broadcast,
                multiply on DVE, DMA out from the GpSimd queue."""
                def f():
                    avs = av_tiles[(pair, hh)]
                    stg = []
                    for w in range(2):
                        a = smallp.tile([HS + 1, HALF], F32, tag="avs",
                                        name=f"avs{pair}_{hh}_{w}")
                        nc.vector.tensor_copy(a[:, :], avs[w][:, :])
                        stg.append(a)
                    rbs = []
                    for w in range(2):
                        den_t = smallp.tile([P, 4], F32, tag="dent",
                                            name=f"den{pair}_{hh}_{w}")
                        nc.sync.dma_start(den_t[:, :], stg[w][HS:HS + 1, :])
                        rec_t = smallp.tile([P, 4], F32, tag="rect",
                                            name=f"rec{pair}_{hh}_{w}")
                        nc.vector.reciprocal(rec_t[:, :], den_t[:, :])
                        rr = smallp.tile([1, HALF], F32, tag="rr",
                                         name=f"rr{pair}_{hh}_{w}")
                        nc.sync.dma_start(rr[:, :], rec_t[:, :])
                        rb = smallp.tile([HS, HALF], F32, tag="rb",
                                         name=f"rb{pair}_{hh}_{w}")
                        nc.gpsimd.partition_broadcast(rb[:, :], rr[0:1, :])
                        rbs.append(rb)
                    for w in range(2):
                        h = 2 * pair + w
                        osb = smallp.tile([HS, HALF], F32, tag="osb",
                                          name=f"osb{pair}_{hh}_{w}")
                        nc.vector.tensor_tensor(
                            osb[:, :], stg[w][0:HS, :], rbs[w][:, :], MULT)
                        nc.gpsimd.dma_start(
                            out[h, :, ds(HALF * hh, HALF)], osb[:, :])
                return f

            # ---- slot 0: QK proj pair0, then scores(0) with V as filler ----
            for _, u in proj_units(0, 0) + proj_units(0, 1):
                u()
            for i in range(TT):
                scores_tile(0, i)
                v_unit(i)()
            for _, u in proj_units(1, 0) + proj_units(1, 1):
                u()

            # ---- slots 1..7 ----
            for s in range(1, PAIRS):
                if s + 2 < PAIRS:
                    load_w(s + 2, 0)
                    load_w(s + 2, 1)
                fill = deque()
                if s + 1 < PAIRS:
                    fill.extend(proj_units(s + 1, 0))
                fill.append((1.1, av_unit(s - 1, 0)))
                fill.append((0.05, norm_unit(s - 1, 0)))
                if s + 1 < PAIRS:
                    fill.extend(proj_units(s + 1, 1))
                fill.append((2.8, av_unit(s - 1, 1)))
                fill.append((0.05, norm_unit(s - 1, 1)))
                for i in range(TT):
                    scores_tile(s, i)
                    if s == PAIRS - 1 and i == 5:
                        # last pair: its first-half AV only needs es tiles
                        # 0..3, all emitted by now — overlap it with the
                        # remaining exp stream instead of the drain.
                        av_unit(s, 0)()
                        norm_unit(s, 0)()
                    budget = _exp_us(i)
                    while fill and budget > 0:
                        cost, u = fill.popleft()
                        u()
                        budget -= cost
                while fill:
                    fill.popleft()[1]()

            # ---- drain: last pair's second half ----
            av_unit(PAIRS - 1, 1)()
            norm_unit(PAIRS - 1, 1)()

    nc.compile()
    return nc


def get_nc():
    global _BUILT
    if _BUILT is None:
        _BUILT = build_nc()
    return _BUILT


def prep_inputs(x, Wq, Wk, Wv):
    """Host-side shard + layout prep. Returns in_maps (one dict per core)."""
    x = np.asarray(x, dtype=np.float32)
    Wq = np.asarray(Wq, dtype=np.float32)
    Wk = np.asarray(Wk, dtype=np.float32)
    Wv = np.asarray(Wv, dtype=np.float32)
    bf = ml_dtypes.bfloat16

    # xT[b]: [C, T] -> [p, c, t] with row 128c+p
    xts = []
    for b in range(B):
        xT = np.ascontiguousarray(x[b].T)          # [C, T]
        xts.append(xT.reshape(CK, P, T).transpose(1, 0, 2).astype(bf))

    def pack_pairs(W):
        # [H, C, hs] -> [pair, C, 128] -> [pair, p, c, f]
        Wp = W.reshape(PAIRS, 2, C, HS).transpose(0, 2, 1, 3).reshape(PAIRS, C, P)
        return Wp.reshape(PAIRS, CK, P, P).transpose(0, 2, 1, 3)  # [pair, p, c, f]

    wq_p = pack_pairs(Wq)
    wk_p = pack_pairs(Wk)
    wqk_host = np.stack([wq_p, wk_p], axis=0).astype(bf)  # [2, pair, p, c, f]
    # wv: [p, c, pair, f]
    wv_host = np.ascontiguousarray(pack_pairs(Wv).transpose(1, 2, 0, 3)).astype(bf)

    return [
        {"xt": np.ascontiguousarray(xts[b]), "wqk": wqk_host, "wv": wv_host}
        for b in range(B)
    ]


def run_on_device(in_maps, **kwargs):
    nc = get_nc()
    return run_bass_kernel_spmd(nc, in_maps, list(range(B)), **kwargs)


def assemble(core_out):
    """[H, HS, T] out^T -> [T, H*HS]: pure layout transpose."""
    return np.ascontiguousarray(core_out.transpose(2, 0, 1).reshape(T, H * HS))


def kernel(x, Wq, Wk, Wv):
    in_maps = prep_inputs(x, Wq, Wk, Wv)
    res = run_on_device(in_maps)
    return np.stack([assemble(res.results[b]["out"]) for b in range(B)], axis=0)


# revision 14
# speedup vs baseline: 1.3936x; 1.0013x over previous
"""Multi-head causal attention (B=8, T=1024, C=1024, H=16, hs=64) on 8 trn2 cores.

Data-parallel over batch: core b computes full attention for x[b].

Device algorithm (per core), all matmuls bf16 inputs / fp32 PSUM accum:
  - xT [C, T] resident in SBUF as 8 per-chunk tiles (fine-grained DMA deps).
  - v computed for all heads: v_all[s, head, s_tile, 0:64] + ones column at 64
    so the AV matmul also produces softmax denominators.
  - per head-pair: qT, kT = W^T @ xT -> [128, T] (2 heads x 64 stacked).
  - scores transposed scT[s_tile, t] = kT_chunk^T @ qT, both heads concurrent
    in PE row groups (0,0)/(64,0); exp fused on ScalarE with ONE activation
    per span covering both heads (3D APs into a merged [128, 2, T] exp tile);
    diagonal 128x128 masked by one two-head tril multiply on VectorE.
  - out^T[65, t] accumulated over s chunks per head (lhsT = [v | 1]); row 64 is
    the softmax denominator. Normalize: DVE fast-reciprocal of the PSUM
    denominator row -> GpSimd partition-broadcast -> one DVE multiply reading
    the AV PSUM directly -> DMA out (issued from the GpSimd queue).

Scheduling: per-engine queues execute in emission order, so the program is
emitted as a software pipeline: slot p interleaves scores(p) (paced against
the ScalarE exp stream via the single-score-PSUM-tile WAR dependency) with
QK-projection chunks of pair p+1 and the AV+normalize of pair p-1 as PE
filler. V-projection chunks fill slot 0. This keeps the PE dense (no HAM
re-throttle) from ~1.5us after launch to the tail.
"""

from collections import deque

import numpy as np
import ml_dtypes

import concourse.bass as bass
import concourse.mybir as mybir
from concourse import bacc
from concourse.tile import TileContext
from concourse.bass import ds, ts
from concourse.bass_utils import run_bass_kernel_spmd
from concourse.masks import make_upper_triangular

BF16 = mybir.dt.bfloat16
F32 = mybir.dt.float32
MULT = mybir.AluOpType.mult
EXP = mybir.ActivationFunctionType.Exp

B, T, C, H, HS = 8, 1024, 1024, 16, 64
P = 128
CK = C // P       # 8 contraction chunks
TT = T // P       # 8 s tiles
PAIRS = H // 2    # 8 head pairs
HALF = 512

_BUILT = None


def _spans(i):
    t0 = P * i
    return [(t0, HALF), (HALF, T)] if t0 < HALF else [(t0, T)]


def _exp_us(i):
    """ScalarE time (us) for tile i's exp instructions (pacing budget)."""
    return sum(2 * (b - a) + 352 for a, b in _spans(i)) / 1200.0


def build_nc():
    nc = bacc.Bacc("TRN2", target_bir_lowering=False, debug=False)
    # [p, c, t] : xT[C, T] chunked; partition p, chunk c -> row 128c+p of xT
    xt = nc.dram_tensor("xt", [P, CK, T], BF16, kind="ExternalInput")
    # [proj(q,k), pair, p, c, f] : lhsT chunks, f = 2 heads x 64 stacked
    wqk = nc.dram_tensor("wqk", [2, PAIRS, P, CK, P], BF16, kind="ExternalInput")
    # [p, c, pair, f]
    wv = nc.dram_tensor("wv", [P, CK, PAIRS, P], BF16, kind="ExternalInput")
    # out^T per head: [head, d, t]; host transposes to [T, H*HS]
    out = nc.dram_tensor("out", [H, HS, T], F32, kind="ExternalOutput")

    with TileContext(nc) as tc:
        with (
            tc.tile_pool(name="const", bufs=1) as constp,
            tc.tile_pool(name="wpool", bufs=8) as wpool,
            tc.tile_pool(name="qkpool", bufs=6) as qkp,
            tc.tile_pool(name="exppool", bufs=18) as expp,
            tc.tile_pool(name="smallpool", bufs=4) as smallp,
            tc.tile_pool(name="psProj", bufs=2, space="PSUM") as psProj,
            tc.tile_pool(name="psSc", bufs=2, space="PSUM") as psSc,
            tc.tile_pool(name="psAV", bufs=2, space="PSUM") as psAV,
        ):
            # ---- input DMAs (sync queue, ordered for earliest PE start) ----
            wq_sb = [None] * PAIRS
            wk_sb = [None] * PAIRS

            def load_w(pair, which):
                t = wpool.tile([P, CK, P], BF16, tag="w",
                               name=f"w{'qk'[which]}{pair}")
                nc.sync.dma_start(t[:, :, :], wqk[which, pair, :, :, :])
                (wq_sb if which == 0 else wk_sb)[pair] = t

            load_w(0, 0)
            # wv rides the Scalar engine's DMA queue so the xt/w loads on
            # the Sync queue aren't serialized behind it.
            wv_c = []
            for c in range(CK):
                t = constp.tile([P, 2, 4 * P], BF16, tag=f"wv{c}", name=f"wv{c}")
                nc.scalar.dma_start(
                    t[:, :, :],
                    wv[:, c, :, :].rearrange("p (g r) f -> p g (r f)", g=2),
                )
                wv_c.append(t)
            xt_c = []
            for c in range(CK):
                t = constp.tile([P, T], BF16, tag=f"xt{c}", name=f"xt{c}")
                nc.sync.dma_start(t[:, :], xt[:, c, :])
                xt_c.append(t)
                if c == 2:
                    load_w(0, 1)
            load_w(1, 0)
            load_w(1, 1)
            load_w(2, 0)
            load_w(2, 1)

            # two-head tril mask [p, w, t'] and v (+ones) for all heads
            mask2 = constp.tile([P, 2, P], BF16, tag="mask", name="mask2")
            for w in range(2):
                make_upper_triangular(nc, mask2[:, w, :], val=1.0, diag=True)
            v_all = constp.tile([P, H, TT, HS + 1], BF16, tag="vall", name="v_all")
            nc.gpsimd.memset(v_all[:, :, :, HS:HS + 1], 1.0)

            qT = [None] * PAIRS
            kT = [None] * PAIRS
            es_tiles = [[None] * TT for _ in range(PAIRS)]
            av_tiles = {}

            # ---- emission units ----
            def proj_units(pair, which):
                """QK projection for one proj: 8 c-units (LDW + 2 MMs) + cast."""
                holder = {}

                def mm(c):
                    if c == 0:
                        wsb = (wq_sb if which == 0 else wk_sb)[pair]
                        dst = qkp.tile([P, T], BF16, tag="qk",
                                       name=f"{'qk'[which]}T{pair}")
                        (qT if which == 0 else kT)[pair] = dst
                        holder['w'] = wsb
                        holder['dst'] = dst
                        holder['ps'] = [
                            psProj.tile([P, HALF], F32, tag="ps",
                                        name=f"p{'qk'[which]}{pair}_{g}")
                            for g in range(2)
                        ]
                    for g in range(2):
                        nc.tensor.matmul(
                            holder['ps'][g][:, :],
                            holder['w'][:, c, :],
                            xt_c[c][:, ds(HALF * g, HALF)],
                            start=(c == 0),
                            stop=(c == CK - 1),
                        )

                def cast():
                    for g in range(2):
                        nc.vector.tensor_copy(
                            holder['dst'][:, ds(HALF * g, HALF)],
                            holder['ps'][g][:, :])

                return [(0.45, lambda c=c: mm(c)) for c in range(CK)] + \
                       [(0.02, cast)]

            def v_unit(j):
                def f():
                    pvs = [psProj.tile([P, HALF], F32, tag="ps",
                                       name=f"pv{j}_{g}") for g in range(2)]
                    for c in range(CK):
                        for pg in range(2):
                            nc.tensor.matmul(
                                pvs[pg][:, :],
                                xt_c[c][:, ts(j, P)],
                                wv_c[c][:, pg, :],
                                start=(c == 0),
                                stop=(c == CK - 1),
                            )
                    for pg in range(2):
                        nc.vector.tensor_copy(
                            v_all[:, ds(8 * pg, 8), j, 0:HS],
                            pvs[pg].rearrange("p (g d) -> p g d", d=HS),
                        )
                return f

            def scores_tile(pair, i):
                t0 = P * i
                es = expp.tile([P, 2, T], BF16, tag="exp", name=f"es{pair}_{i}")
                es_tiles[pair][i] = es
                sc = psSc.tile([P, T], F32, tag="sc", name=f"sc{pair}_{i}")
                scv = sc.rearrange("p (w t) -> p w t", w=2)
                for a, b in _spans(i):
                    L = b - a
                    for w in range(2):
                        nc.tensor.matmul(
                            scv[:, w, 0:L],
                            kT[pair][ds(HS * w, HS), ds(t0, P)],
                            qT[pair][ds(HS * w, HS), ds(a, L)],
                        )
                    nc.scalar.activation(
                        es[:, :, ds(a, L)], scv[:, :, 0:L], EXP,
                        scale=HS ** -0.5,
                    )
                    if a == t0:  # diagonal block lives in the first span
                        nc.vector.tensor_tensor(
                            es[:, :, ds(t0, P)], es[:, :, ds(t0, P)],
                            mask2[:, :, :], MULT,
                        )

            def av_unit(pair, hh, ws=(0, 1)):
                def f():
                    avs = {w: psAV.tile([HS + 1, HALF], F32, tag="av",
                                        name=f"av{pair}_{hh}_{w}") for w in ws}
                    for w in ws:
                        av_tiles[(pair, hh, w)] = avs[w]
                    contrib = [i for i in range(TT) if P * i < HALF * (hh + 1)]
                    for idx, i in enumerate(contrib):
                        g0 = max(HALF * hh, P * i)
                        g1 = HALF * (hh + 1)
                        for w in ws:
                            nc.tensor.matmul(
                                avs[w][:, ds(g0 - HALF * hh, g1 - g0)],
                                v_all[:, 2 * pair + w, i, :],
                                es_tiles[pair][i][:, w, ds(g0, g1 - g0)],
                                start=(idx == 0),
                                stop=(idx == len(contrib) - 1),
                            )
                return f

            def norm_unit(pair, hh, ws=(0, 1)):
                """Stage AV PSUM out fast (frees the psAV slot), then
                normalize off SBUF: DMA-repartition the [1,512] denominator
                row to [128,4] (DVE reciprocal cost scales with per-lane
                free size), recip, DMA back, GpSimd partition-broadcast,
                multiply on DVE, DMA out from the GpSimd queue."""
                def f():
                    stg, rbs = {}, {}
                    for w in ws:
                        a = smallp.tile([HS + 1, HALF], F32, tag="avs",
                                        name=f"avs{pair}_{hh}_{w}")
                        nc.vector.tensor_copy(a[:, :], av_tiles[(pair, hh, w)])
                        stg[w] = a
                    for w in ws:
                        den_t = smallp.tile([P, 4], F32, tag="dent",
                                            name=f"den{pair}_{hh}_{w}")
                        nc.sync.dma_start(den_t[:, :], stg[w][HS:HS + 1, :])
                        rec_t = smallp.tile([P, 4], F32, tag="rect",
                                            name=f"rec{pair}_{hh}_{w}")
                        nc.vector.reciprocal(rec_t[:, :], den_t[:, :])
                        rr = smallp.tile([1, HALF], F32, tag="rr",
                                         name=f"rr{pair}_{hh}_{w}")
                        nc.sync.dma_start(rr[:, :], rec_t[:, :])
                        rb = smallp.tile([HS, HALF], F32, tag="rb",
                                         name=f"rb{pair}_{hh}_{w}")
                        nc.gpsimd.partition_broadcast(rb[:, :], rr[0:1, :])
                        rbs[w] = rb
                    for w in ws:
                        h = 2 * pair + w
                        osb = smallp.tile([HS, HALF], F32, tag="osb",
                                          name=f"osb{pair}_{hh}_{w}")
                        nc.vector.tensor_tensor(
                            osb[:, :], stg[w][0:HS, :], rbs[w][:, :], MULT)
                        nc.gpsimd.dma_start(
                            out[h, :, ds(HALF * hh, HALF)], osb[:, :])
                return f

            # ---- slot 0: QK proj pair0, then scores(0) with V as filler ----
            for _, u in proj_units(0, 0) + proj_units(0, 1):
                u()
            for i in range(TT):
                scores_tile(0, i)
                v_unit(i)()
            for _, u in proj_units(1, 0) + proj_units(1, 1):
                u()

            # ---- slots 1..7 ----
            for s in range(1, PAIRS):
                if s + 2 < PAIRS:
                    load_w(s + 2, 0)
                    load_w(s + 2, 1)
                fill = deque()
                if s + 1 < PAIRS:
                    fill.extend(proj_units(s + 1, 0))
                fill.append((1.1, av_unit(s - 1, 0)))
                fill.append((0.05, norm_unit(s - 1, 0)))
                if s + 1 < PAIRS:
                    fill.extend(proj_units(s + 1, 1))
                fill.append((2.8, av_unit(s - 1, 1)))
                fill.append((0.05, norm_unit(s - 1, 1)))
                for i in range(TT):
                    scores_tile(s, i)
                    if s == PAIRS - 1 and i == 5:
                        # last pair: its first-half AV only needs es tiles
                        # 0..3, all emitted by now — overlap it with the
                        # remaining exp stream instead of the drain.
                        av_unit(s, 0)()
                        norm_unit(s, 0)()
                    budget = _exp_us(i)
                    while fill and budget > 0:
                        cost, u = fill.popleft()
                        u()
                        budget -= cost
                while fill:
                    fill.popleft()[1]()

            # ---- drain: last pair's second half, per head to shorten the
            # exposed normalize chain ----
            av_unit(PAIRS - 1, 1, ws=(0,))()
            norm_unit(PAIRS - 1, 1, ws=(0,))()
            av_unit(PAIRS - 1, 1, ws=(1,))()
            norm_unit(PAIRS - 1, 1, ws=(1,))()

    nc.compile()
    return nc


def get_nc():
    global _BUILT
    if _BUILT is None:
        _BUILT = build_nc()
    return _BUILT


def prep_inputs(x, Wq, Wk, Wv):
    """Host-side shard + layout prep. Returns in_maps (one dict per core)."""
    x = np.asarray(x, dtype=np.float32)
    Wq = np.asarray(Wq, dtype=np.float32)
    Wk = np.asarray(Wk, dtype=np.float32)
    Wv = np.asarray(Wv, dtype=np.float32)
    bf = ml_dtypes.bfloat16

    # xT[b]: [C, T] -> [p, c, t] with row 128c+p
    xts = []
    for b in range(B):
        xT = np.ascontiguousarray(x[b].T)          # [C, T]
        xts.append(xT.reshape(CK, P, T).transpose(1, 0, 2).astype(bf))

    def pack_pairs(W):
        # [H, C, hs] -> [pair, C, 128] -> [pair, p, c, f]
        Wp = W.reshape(PAIRS, 2, C, HS).transpose(0, 2, 1, 3).reshape(PAIRS, C, P)
        return Wp.reshape(PAIRS, CK, P, P).transpose(0, 2, 1, 3)  # [pair, p, c, f]

    wq_p = pack_pairs(Wq)
    wk_p = pack_pairs(Wk)
    wqk_host = np.stack([wq_p, wk_p], axis=0).astype(bf)  # [2, pair, p, c, f]
    # wv: [p, c, pair, f]
    wv_host = np.ascontiguousarray(pack_pairs(Wv).transpose(1, 2, 0, 3)).astype(bf)

    return [
        {"xt": np.ascontiguousarray(xts[b]), "wqk": wqk_host, "wv": wv_host}
        for b in range(B)
    ]


def run_on_device(in_maps, **kwargs):
    nc = get_nc()
    return run_bass_kernel_spmd(nc, in_maps, list(range(B)), **kwargs)


def assemble(core_out):
    """[H, HS, T] out^T -> [T, H*HS]: pure layout transpose."""
    return np.ascontiguousarray(core_out.transpose(2, 0, 1).reshape(T, H * HS))


def kernel(x, Wq, Wk, Wv):
    in_maps = prep_inputs(x, Wq, Wk, Wv)
    res = run_on_device(in_maps)
    return np.stack([assemble(res.results[b]["out"]) for b in range(B)], axis=0)
